# revision 26
# baseline (speedup 1.0000x reference)
"""Expert-parallel MoE (Kimi/DeepSeek-V3 sparse block) on 8 trn2 NeuronCores.

v2 strategy — uniform F=512 chunk pipeline with 2-way expert split:
  - Host computes the sigmoid gate + group-limited top-2 routing in float64.
  - Every unit of device work is a uniform "chunk": n<=512 tokens through a
    SwiGLU FFN with intermediate F=512 (w1/w3 up, w2 down), k-outer over the
    8 D-tiles so weights+x stream at k-pair granularity.
  - The 4096-token shared expert (SH=1024) is exactly two such F=512 chunks
    per 512-token slice (halves of the SH dim; outputs add on the host).
  - Routed experts are split 2-way: experts sorted by load; the top-4 are
    "A" experts (halves on two cores' A slots, capacity RA=ceil(max/2)),
    the bottom-4 are "B" experts (capacity RB). Per-core capacity is
    RA+RB+2*512 ~ 2104 token-units instead of max_load+1024 ~ 2192.
  - Per core chunk order: A0 (weights+x host-packed into one streamed
    tensor), B, SH-half0, SH-half1, A-remainder (smallest chunk last so the
    post-matmul tail is short).
  - All matmul operands bf16 (fp32 PSUM); every DMA >= 1KB contiguous per
    partition; input descriptors round-robin over the 3 DMA-capable queues
    (sync/gpsimd/scalar) so the first k=0 slices land ~3us after the
    framework preamble; 3 zero-tile warmup matmuls bridge the gap and start
    the PE HAM clock ramp.
  - Up-phase tail (k=KD-1) interleaves h1/h3 per mf with immediate
    silu+mul so the down phase starts with minimal PE bubble; down phase in
    two md-halves with per-half output DMAs; the final chunk writes a
    compact [P, KD*n] output flushed on the sync queue.
"""

from contextlib import ExitStack

import numpy as np
import ml_dtypes

import concourse.bacc as bacc
import concourse.tile as tile
import concourse.mybir as mybir
from concourse import bass_utils

# --- model dims (hardcoded per problem spec) ---
B, S, D = 2, 2048, 1024
T = B * S                 # 4096 tokens
E, F = 8, 512             # routed experts / expert intermediate
SH = 1024                 # shared intermediate (= 2 * F)
TOP_K, N_GROUP, TOPK_GROUP = 2, 4, 2
SCALE = 2.5

N_CORES = 8
P = 128                   # SBUF partitions
KD = D // P               # 8 contraction tiles over D
KF = F // P               # 4 F-tiles per chunk
NT = 512                  # max matmul moving free dim (one PSUM bank fp32)
SHT = T // N_CORES        # 512 shared-expert tokens per core
RA_MAX = 2048             # capacity ceilings (SBUF); overflow -> host
RB_MAX = 1024
N_WARM = 40               # N=64 HAM warm-up matmuls (bridge preamble->data)

F32 = mybir.dt.float32
BF16 = mybir.dt.bfloat16
BF16_NP = ml_dtypes.bfloat16

_CACHE: dict = {}


def _al8(v):
    return -(-int(v) // 8) * 8


def _chunk_plan(RA, RB):
    """Uniform chunks: (mode, col_off, ncols); modes a/b/s0/s1.

    a0 first (host-packed streaming tensor), then b, the two shared
    halves, and the small a-remainder last."""
    a = [("a", c, min(NT, RA - c)) for c in range(0, RA, NT)]
    b = [("b", c, min(NT, RB - c)) for c in range(0, RB, NT)]
    s = [("s0", c, min(NT, SHT - c)) for c in range(0, SHT, NT)] + \
        [("s1", c, min(NT, SHT - c)) for c in range(0, SHT, NT)]
    return tuple(a[:1] + b + s + a[1:])


def _emit(nc, RA, RB):
    chunks = _chunk_plan(RA, RB)
    n0 = min(NT, RA)
    W0 = 2 * F + n0    # packed a0 row: w1 | x0 | w3 per k (w1+x contiguous
    XO = F             # so ONE descriptor feeds the first h1 matmuls)
    W3O = F + n0

    s0d = nc.dram_tensor("s0pack", [P, KD, W0], BF16, kind="ExternalInput").ap()
    xrd = (nc.dram_tensor("xrda", [P, KD, RA - n0], BF16, kind="ExternalInput").ap()
           if RA > n0 else None)
    wbd = (nc.dram_tensor("wbpk", [P, KD, 2 * F], BF16, kind="ExternalInput").ap()
           if RB else None)
    xbd = (nc.dram_tensor("xb", [P, KD, RB], BF16, kind="ExternalInput").ap()
           if RB else None)
    wsd = [nc.dram_tensor(f"wsh{h}", [P, KD, 2 * F], BF16,
                          kind="ExternalInput").ap() for h in range(2)]
    xshd = nc.dram_tensor("xshd", [P, KD, SHT], BF16, kind="ExternalInput").ap()
    w2modes = ["a"] + (["b"] if RB else []) + ["s0", "s1"]
    w2d = {m: nc.dram_tensor(f"w2_{m}", [P, KF, D], BF16,
                             kind="ExternalInput").ap() for m in w2modes}
    # a small last chunk uses a compact [P, KD*n] layout so its final flush
    # is contiguous per partition (tail-optimized path in ffn below)
    tail_opt = chunks[-1][2] <= NT // 2
    youts = [nc.dram_tensor(f"y{j}", ([P, KD * n] if j == len(chunks) - 1
                                      and tail_opt else [P, KD, n]),
                            BF16, kind="ExternalOutput").ap()
             for j, (mode, c0, n) in enumerate(chunks)]

    silu = mybir.ActivationFunctionType.Silu

    with tile.TileContext(nc) as tc, ExitStack() as ctx:
        wpool = ctx.enter_context(tc.tile_pool(name="wpool", bufs=1))
        xpool = ctx.enter_context(tc.tile_pool(name="xpool", bufs=1))
        hpool = ctx.enter_context(tc.tile_pool(name="hpool", bufs=2))
        opool = ctx.enter_context(tc.tile_pool(name="opool", bufs=2))
        pspool = ctx.enter_context(tc.tile_pool(name="pspool", bufs=1, space="PSUM"))

        ps_tags = [f"ps{i}" for i in range(8)]

        # ---- SBUF tiles
        s0p = [wpool.tile([P, 2, W0], BF16, name=f"s0p{j}") for j in range(KD // 2)]
        wbt = wpool.tile([P, KD, 2 * F], BF16, name="wbt") if RB else None
        wst = [wpool.tile([P, KD, 2 * F], BF16, name=f"wst{h}") for h in range(2)]
        w2t = {m: wpool.tile([P, KF, D], BF16, name=f"w2t_{m}") for m in w2modes}
        xsh = xpool.tile([P, KD, SHT], BF16, name="xsh")
        xbt = xpool.tile([P, KD, RB], BF16, name="xbt") if RB else None
        xrest = (xpool.tile([P, KD, RA - n0], BF16, name="xrest")
                 if RA > n0 else None)

        # ---- HAM warm-up: short N=64 matmuls during the initial DMA wait
        # keep the PE busy from right after the preamble (the gpsimd memset
        # of a [P,64] slice is the cheapest cross-engine dependency) so the
        # clock gate is 8/8 by the time real data lands.
        wu = xpool.tile([P, 64], BF16, name="wu")
        nc.gpsimd.memset(wu[:], 0)
        wups = pspool.tile([P, NT], F32, name="wups", tag=ps_tags[7])
        for _ in range(N_WARM):
            nc.tensor.matmul(wups[0:64, 0:64], wu[:], wu[:],
                             start=True, stop=True)

        # ---- input DMA stream: sync queue ONLY (a single HW ring bursts
        # ~350GB/s; concurrent rings time-share down to ~160GB/s
        # aggregate), in chunk-consumption order; x/w k-pairs interleaved
        # so each chunk's k=0 data lands well before its first matmul
        dma = nc.sync.dma_start
        # k0/k1 at single-row granularity, w1+x ahead of w3, so the first
        # h1 matmuls are gated by one 0.75us transfer instead of three
        for r in range(2):
            dma(s0p[0][:, r, 0:W3O], s0d[:, r, 0:W3O])    # a0 k{r} w1+x
            dma(s0p[0][:, r, W3O:W0], s0d[:, r, W3O:W0])  # a0 k{r} w3
        for j in range(1, KD // 2):
            dma(s0p[j][:, :, 0:W3O], s0d[:, 2 * j:2 * j + 2, 0:W3O])
            dma(s0p[j][:, :, W3O:W0], s0d[:, 2 * j:2 * j + 2, W3O:W0])
        dma(w2t["a"][:], w2d["a"][:])
        if RB:
            for j in range(KD // 2):
                dma(xbt[:, 2 * j:2 * j + 2, :], xbd[:, 2 * j:2 * j + 2, :])
                dma(wbt[:, 2 * j:2 * j + 2, :], wbd[:, 2 * j:2 * j + 2, :])
            dma(w2t["b"][:], w2d["b"][:])
        for j in range(KD // 2):
            dma(xsh[:, 2 * j:2 * j + 2, :], xshd[:, 2 * j:2 * j + 2, :])
            dma(wst[0][:, 2 * j:2 * j + 2, :], wsd[0][:, 2 * j:2 * j + 2, :])
        dma(w2t["s0"][:], w2d["s0"][:])
        for j in range(KD // 2):
            dma(wst[1][:, 2 * j:2 * j + 2, :], wsd[1][:, 2 * j:2 * j + 2, :])
        dma(w2t["s1"][:], w2d["s1"][:])
        if xrest is not None:
            dma(xrest[:], xrd[:])

        last_ci = len(chunks) - 1

        def ffn(ci, mode, c0, n):
            yj = youts[ci]
            w2 = w2t[mode]
            wsrc = {"b": wbt, "s0": wst[0], "s1": wst[1]}.get(mode)

            def wsl(which, k, mf):
                if mode == "a":
                    off = (0 if which == 1 else W3O) + mf * P
                    return s0p[k // 2][:, k % 2, off:off + P]
                off = (0 if which == 1 else F) + mf * P
                return wsrc[:, k, off:off + P]

            def xsl(k):
                if mode == "a":
                    if ci == 0:
                        return s0p[k // 2][:, k % 2, XO:XO + n]
                    return xrest[:, k, c0 - n0:c0 - n0 + n]
                if mode == "b":
                    return xbt[:, k, c0:c0 + n]
                return xsh[:, k, c0:c0 + n]

            h1s, h3s, hts = [None] * KF, [None] * KF, [None] * KF

            def act_mul(mf):
                a = hpool.tile([P, NT], F32, name="asb", tag="silu")
                nc.scalar.activation(a[:, 0:n], h1s[mf][:, 0:n], silu)
                ht = hpool.tile([P, NT], BF16, name="htsb", tag=f"ht{mf}")
                nc.vector.tensor_mul(ht[:, 0:n], a[:, 0:n], h3s[mf][:, 0:n])
                hts[mf] = ht

            copy_eng = [nc.vector.tensor_copy, nc.scalar.copy]
            if ci == last_ci and tail_opt:
                # tail-optimized last chunk: mf-outer up on ps0-3 only (so
                # it never waits on the previous chunk's ps4-7 copy-out),
                # paired-PSUM down (two md's share a bank) for 4 copies,
                # and the two half flushes ride parallel rings.
                for mf in range(KF):
                    h1s[mf] = pspool.tile([P, NT], F32, name="h1ps",
                                          tag=ps_tags[2 * (mf % 2)])
                    for k in range(KD):
                        nc.tensor.matmul(h1s[mf][:, 0:n], wsl(1, k, mf),
                                         xsl(k), start=(k == 0),
                                         stop=(k == KD - 1))
                    h3s[mf] = pspool.tile([P, NT], F32, name="h3ps",
                                          tag=ps_tags[2 * (mf % 2) + 1])
                    for k in range(KD):
                        nc.tensor.matmul(h3s[mf][:, 0:n], wsl(3, k, mf),
                                         xsl(k), start=(k == 0),
                                         stop=(k == KD - 1))
                    act_mul(mf)
                ysb = opool.tile([P, KD * n], BF16, name="ysbc", tag="ysbc")
                for half in range(2):
                    prs = [pspool.tile([P, 2 * n], F32, name="ypr",
                                       tag=ps_tags[4 + 2 * half + q])
                           for q in range(2)]
                    for kf in range(KF):
                        for q in range(2):
                            for s in range(2):
                                # ONE start/stop per PSUM bank: start=True
                                # clears has_written for the whole bank, so
                                # only the bank's first matmul may set it
                                md = half * 4 + 2 * q + s
                                nc.tensor.matmul(
                                    prs[q][:, s * n:(s + 1) * n],
                                    w2[:, kf, md * P:(md + 1) * P],
                                    hts[kf][:, 0:n],
                                    start=(kf == 0 and s == 0),
                                    stop=(kf == KF - 1 and s == 1))
                    for q in range(2):
                        lo = (half * 4 + 2 * q) * n
                        copy_eng[q % 2](ysb[:, lo:lo + 2 * n], prs[q][:])
                    lo = half * 4 * n
                    eng = nc.sync if half == 0 else nc.scalar
                    eng.dma_start(yj[:, lo:lo + 4 * n],
                                  ysb[:, lo:lo + 4 * n])
                return

            # up phase, k-outer (weights/x consumed in DMA arrival order);
            # the last k-group interleaves h1/h3 per mf with immediate
            # act+mul so hts[0..3] are ready right as the up phase ends
            for mf in range(KF):
                h1s[mf] = pspool.tile([P, NT], F32, name="h1ps", tag=ps_tags[2 * mf])
                h3s[mf] = pspool.tile([P, NT], F32, name="h3ps", tag=ps_tags[2 * mf + 1])
            for k in range(KD - 1):
                xs = xsl(k)
                st = (k == 0)
                if st and ci > 0:
                    # interleave h1/h3 pairs at k=0: the first MMs touch
                    # PSUM banks already freed by the previous chunk's
                    # down-half0 copies, hiding the half1-copy handoff
                    for mf in range(KF):
                        nc.tensor.matmul(h1s[mf][:, 0:n], wsl(1, k, mf), xs,
                                         start=True, stop=False)
                        nc.tensor.matmul(h3s[mf][:, 0:n], wsl(3, k, mf), xs,
                                         start=True, stop=False)
                    continue
                for mf in range(KF):
                    nc.tensor.matmul(h1s[mf][:, 0:n], wsl(1, k, mf), xs,
                                     start=st, stop=False)
                for mf in range(KF):
                    nc.tensor.matmul(h3s[mf][:, 0:n], wsl(3, k, mf), xs,
                                     start=st, stop=False)
            xs = xsl(KD - 1)
            for mf in range(KF):
                nc.tensor.matmul(h1s[mf][:, 0:n], wsl(1, KD - 1, mf), xs,
                                 start=False, stop=True)
                nc.tensor.matmul(h3s[mf][:, 0:n], wsl(3, KD - 1, mf), xs,
                                 start=False, stop=True)
                act_mul(mf)

            # down-phase in two md-halves (kf-outer inside each) so the
            # first half's outputs flush while the second half computes
            ysb = opool.tile([P, KD, NT], BF16, name="ysb", tag="ysb")
            H = KD // 2
            for half in range(2):
                mds = range(half * H, (half + 1) * H)
                yps = {md: pspool.tile([P, NT], F32, name="yps", tag=ps_tags[md])
                       for md in mds}
                for kf in range(KF):
                    st, sp = (kf == 0), (kf == KF - 1)
                    for md in mds:
                        nc.tensor.matmul(yps[md][:, 0:n],
                                         w2[:, kf, md * P:(md + 1) * P],
                                         hts[kf][:, 0:n], start=st, stop=sp)
                hsl = slice(half * H, (half + 1) * H)
                for md in mds:
                    copy_eng[md % 2](ysb[:, md, 0:n], yps[md][:, 0:n])
                dma_eng = nc.gpsimd if half == 0 else nc.scalar
                dma_eng.dma_start(yj[:, hsl, :], ysb[:, hsl, 0:n])

        for ci, (mode, c0, n) in enumerate(chunks):
            ffn(ci, mode, c0, n)


def _get_nc(RA, RB):
    key = ("nc", RA, RB)
    if key not in _CACHE:
        nc = bacc.Bacc("TRN2", target_bir_lowering=False, debug=False,
                       num_devices=N_CORES)
        _emit(nc, RA, RB)
        nc.compile()
        _CACHE[key] = nc
    return _CACHE[key]


def _gate_numpy(x2d, gate_w, gate_bias):
    """Replicates reference _moe_gate in float64 (routing-stable)."""
    xl = x2d.astype(np.float64)
    logits = xl @ gate_w.astype(np.float64).T
    scores = 1.0 / (1.0 + np.exp(-logits))
    sc = scores + gate_bias.astype(np.float64)[None, :]
    grp = sc.reshape(T, N_GROUP, E // N_GROUP)
    group_scores = np.sort(grp, axis=-1)[:, :, -2:].sum(-1)
    gidx = np.argsort(-group_scores, axis=-1, kind="stable")[:, :TOPK_GROUP]
    gmask = np.zeros((T, N_GROUP), bool)
    gmask[np.arange(T)[:, None], gidx] = True
    smask = np.repeat(gmask, E // N_GROUP, axis=1)
    tmp = np.where(smask, sc, 0.0)
    tidx = np.argsort(-tmp, axis=-1, kind="stable")[:, :TOP_K]
    tw = np.take_along_axis(scores, tidx, axis=1)
    tw = tw / (tw.sum(-1, keepdims=True) + 1e-20)
    return tidx, (tw * SCALE).astype(np.float32)


def _ffn_host(x, w1e, w2e, w3e):
    """Host fallback for capacity-overflow tokens (pathological skew only)."""
    h = x @ w1e.T
    h = (h / (1.0 + np.exp(-h))) * (x @ w3e.T)
    return h @ w2e.T


def _ikp(mat, kt):
    """[kt*P, X] -> [P, kt, X] bf16 (k-interleaved, partition-major)."""
    return np.asarray(mat).reshape(kt, P, -1).transpose(1, 0, 2).astype(BF16_NP)


def kernel(hidden_states, gate_w, gate_bias, w1, w2, w3,
           shared_gate_w, shared_up_w, shared_down_w):
    hidden_states = np.ascontiguousarray(np.asarray(hidden_states, np.float32))
    gate_w = np.asarray(gate_w, np.float32)
    gate_bias = np.asarray(gate_bias, np.float32)
    w1 = np.asarray(w1, np.float32)
    w2 = np.asarray(w2, np.float32)
    w3 = np.asarray(w3, np.float32)
    shared_gate_w = np.asarray(shared_gate_w, np.float32)
    shared_up_w = np.asarray(shared_up_w, np.float32)
    shared_down_w = np.asarray(shared_down_w, np.float32)

    x2d = hidden_states.reshape(T, D)
    tidx, tw = _gate_numpy(x2d, gate_w, gate_bias)

    counts = np.bincount(tidx.ravel(), minlength=E)
    order = [int(e) for e in np.argsort(-counts, kind="stable")]
    RA = max(min(_al8(-(-int(counts[order[0]]) // 2)), RA_MAX), 8)
    RB = min(_al8(-(-int(counts[order[4]]) // 2)), RB_MAX)
    chunks = _chunk_plan(RA, RB)
    n0 = min(NT, RA)

    x2dT = np.ascontiguousarray(x2d.T)  # [D, T]

    # split each expert's tokens into two halves; cap per slot, rest -> host
    overflow = []
    split_e = {}
    for rank, e in enumerate(order):
        rows, slots = np.nonzero(tidx == e)
        w = tw[rows, slots]
        h0 = (len(rows) + 1) // 2
        cap = RA if rank < 4 else RB
        parts = []
        for rr, ww in ((rows[:h0], w[:h0]), (rows[h0:], w[h0:])):
            if len(rr) > cap:
                overflow.append((e, rr[cap:], ww[cap:]))
                rr, ww = rr[:cap], ww[:cap]
            parts.append((rr, ww))
        split_e[e] = parts

    # shared packs (identical on all cores)
    wsh_pk = [np.concatenate([_ikp(shared_gate_w[h * F:(h + 1) * F].T, KD),
                              _ikp(shared_up_w[h * F:(h + 1) * F].T, KD)],
                             axis=2) for h in range(2)]
    w2sh_h = [_ikp(np.ascontiguousarray(shared_down_w.T[h * F:(h + 1) * F]), KF)
              for h in range(2)]

    in_maps, core_meta = [], []
    for i in range(N_CORES):
        ae, be = order[i // 2], order[4 + i // 2]
        rowsA, wtsA = split_e[ae][i % 2]
        rowsB, wtsB = split_e[be][i % 2]
        xa = np.zeros((D, RA), np.float32)
        xa[:, :len(rowsA)] = x2dT[:, rowsA]
        im = {
            "s0pack": np.concatenate([_ikp(w1[ae].T, KD), _ikp(xa[:, :n0], KD),
                                      _ikp(w3[ae].T, KD)], axis=2),
            "w2_a": _ikp(w2[ae].T, KF),
            "wsh0": wsh_pk[0], "wsh1": wsh_pk[1],
            "w2_s0": w2sh_h[0], "w2_s1": w2sh_h[1],
            "xshd": _ikp(x2dT[:, i * SHT:(i + 1) * SHT], KD),
        }
        if RA > n0:
            im["xrda"] = _ikp(xa[:, n0:], KD)
        if RB:
            xb = np.zeros((D, RB), np.float32)
            xb[:, :len(rowsB)] = x2dT[:, rowsB]
            im["wbpk"] = np.concatenate([_ikp(w1[be].T, KD),
                                         _ikp(w3[be].T, KD)], axis=2)
            im["xb"] = _ikp(xb, KD)
            im["w2_b"] = _ikp(w2[be].T, KF)
        in_maps.append(im)
        core_meta.append((rowsA, wtsA, rowsB, wtsB))

    nc = _get_nc(RA, RB)
    res = bass_utils.run_bass_kernel_spmd(
        nc, in_maps, core_ids=list(range(N_CORES))
    )
    _CACHE["last_res"] = res

    y = np.zeros((T, D), np.float32)
    for i in range(N_CORES):
        rowsA, wtsA, rowsB, wtsB = core_meta[i]
        out = res.results[i]
        for j, (mode, c0, nj) in enumerate(chunks):
            # y{j} is [P, KD, nj] (last: compact [P, KD*nj]); row d = md*P + p
            blk = np.asarray(out[f"y{j}"], np.float32).reshape(P, KD, nj)
            blk = blk.transpose(1, 0, 2).reshape(D, nj)
            if mode in ("a", "b"):
                rows, wts = (rowsA, wtsA) if mode == "a" else (rowsB, wtsB)
                lo, hi = c0, min(c0 + nj, len(rows))
                if hi > lo:
                    y[rows[lo:hi]] += wts[lo:hi, None] * blk[:, 0:hi - lo].T
            else:  # shared-half output for this core's token slice
                sl = slice(i * SHT + c0, i * SHT + c0 + nj)
                y[sl] += blk.T
    for e, rows, wts in overflow:
        y[rows] += wts[:, None] * _ffn_host(x2d[rows], w1[e], w2[e], w3[e])

    return y.reshape(B, S, D)
